# revision 11
# baseline (speedup 1.0000x reference)
"""GCN (2x GCNConv + mean-pool + FC + log_softmax) on 8 Trainium2 NeuronCores.

Device does ALL floating-point math: both GCN feature matmuls, degree
normalization (reciprocal/sqrt on-device), every aggregation SUM (strided
48-slot tensor_reduce), self-loop terms, relu, per-graph pooling reductions,
the FC head and log_softmax.

Host does only integer/index work and data marshaling: sharding, edge
bucketing into a per-node 48-slot grid, and the gather *placement* of
device-computed g-rows into that grid between device programs. This split is
forced by the deployment: the Anthropic extended Q7 ucode (ap_gather /
dma_gather / dma_scatter_add) is absent from this image and walrus dynamic
DMA (indirect_dma_start) is disabled, so the hardware exposes no
data-dependent gather/scatter primitive (verified empirically: the extended
instructions no-op or wedge the device). All arithmetic remains on-device.

Four SPMD device programs:
  P1: h1 = x @ W1, dinv = 1/sqrt(deg), g1 = dinv * h1
  P2: agg1 = slot-reduce(msgs1); relu(dinv*(agg1+g1)+b1) @ W2 * dinv -> g2
  P3: agg2 = slot-reduce(msgs2); z = relu(dinv*(agg2+g2)+b2)
  P4: per-graph pooling reduces over globally-sorted z, mean, FC, log_softmax
"""

import os
import sys

if "/opt/trn_rl_repo" not in sys.path:
    sys.path.insert(0, "/opt/trn_rl_repo")

from contextlib import ExitStack

import ml_dtypes
import numpy as np

import concourse.bacc as bacc
import concourse.mybir as mybir
from concourse.bass_utils import run_bass_kernel_spmd

BF16 = ml_dtypes.bfloat16

N_NODES = 100000
D_IN = 128
HID = 64
N_CLS = 10
N_GRAPHS = 512
NC = 8
SH = 12500          # real nodes per core
SHP = 12544         # padded per-core node count
HALF = SHP // 2     # 6272
SLOTS = 48
CHUNK = 224         # nodes per half-group per chunk
NCHUNK = HALF // CHUNK  # 28
CF = CHUNK * SLOTS
GHALF = 50176       # nodes per half-group in P4's global z layout

F32 = mybir.dt.float32
BF = mybir.dt.bfloat16
AX = mybir.AxisListType
OP = mybir.AluOpType
ACT = mybir.ActivationFunctionType

_TRACE_NS = []


def _run(nc, in_maps, label):
    nc.compile()
    trace = bool(os.environ.get("KERNEL_TRACE"))
    res = run_bass_kernel_spmd(nc, in_maps, list(range(NC)), trace=trace)
    if trace and res.exec_time_ns:
        _TRACE_NS.append((label, res.exec_time_ns))
    return res.results


# ---------------------------------------------------------------- P1
def _build_p1():
    nc = bacc.Bacc()
    xT = nc.declare_dram_parameter("xT", [128, SHP], BF, isOutput=False)
    degT = nc.declare_dram_parameter("degT", [64, SHP], F32, isOutput=False)
    w1 = nc.declare_dram_parameter("w1", [128, HID], BF, isOutput=False)
    g1_o = nc.declare_dram_parameter("g1", [64, SHP], BF, isOutput=True)
    dinv_o = nc.declare_dram_parameter("dinv", [64, SHP], BF, isOutput=True)
    NCH = SHP // (CHUNK * 2)  # 28
    with ExitStack() as ctx:
        _n = iter(range(1000))
        sb = lambda s, d: ctx.enter_context(nc.sbuf_tensor(f"t{next(_n)}", s, d))
        x_s = sb([128, SHP], BF)
        deg_s = sb([64, SHP], F32)
        w1_s = sb([128, HID], BF)
        dinv_s = sb([64, SHP], F32)
        dinvb_s = sb([64, SHP], BF)
        g1_s = sb([64, SHP], BF)
        ps = [ctx.enter_context(nc.psum_tensor(f"ps{i}", [64, CHUNK * 2], F32)) for i in range(2)]
        dma = ctx.enter_context(nc.semaphore("dma"))
        vs = ctx.enter_context(nc.semaphore("vs"))
        ts = ctx.enter_context(nc.semaphore("ts"))
        ss = ctx.enter_context(nc.semaphore("ss"))
        blk = ctx.enter_context(nc.Block())

        with nc.allow_low_precision("bf16 dataflow by design"):
            @blk.sync
            def _(e):
                e.dma_start(out=x_s[:], in_=xT[:]).then_inc(dma, 16)
                e.dma_start(out=deg_s[:], in_=degT[:]).then_inc(dma, 16)
                e.dma_start(out=w1_s[:], in_=w1[:]).then_inc(dma, 16)
                e.wait_ge(vs, 2 + NCH)
                e.dma_start(out=g1_o[:], in_=g1_s[:]).then_inc(dma, 16)
                e.dma_start(out=dinv_o[:], in_=dinvb_s[:]).then_inc(dma, 16)
                e.wait_ge(dma, 16 * 5)

            @blk.vector
            def _(e):
                e.wait_ge(dma, 32)
                # dinv2 = 1/deg (in place chain: dinv_s holds 1/deg)
                e.reciprocal(dinv_s[:], deg_s[:]).then_inc(vs, 1)
                e.wait_ge(ss, 1)  # scalar sqrt done -> dinv_s = 1/sqrt(deg)
                e.tensor_copy(out=dinvb_s[:], in_=dinv_s[:]).then_inc(vs, 1)
                for c in range(NCH):
                    e.wait_ge(ts, c + 1)
                    sl = slice(c * CHUNK * 2, (c + 1) * CHUNK * 2)
                    e.tensor_tensor(
                        out=g1_s[:, sl], in0=ps[c % 2][:], in1=dinv_s[:, sl],
                        op=OP.mult,
                    ).then_inc(vs, 1)

            @blk.scalar
            def _(e):
                e.wait_ge(vs, 1)
                e.activation(dinv_s[:], dinv_s[:], ACT.Sqrt).then_inc(ss, 1)

            @blk.tensor
            def _(e):
                e.wait_ge(dma, 48)
                e.wait_ge(ss, 1)
                for c in range(NCH):
                    if c >= 2:
                        e.wait_ge(vs, 2 + c - 1)  # psum WAR
                    sl = slice(c * CHUNK * 2, (c + 1) * CHUNK * 2)
                    nc.tensor.matmul(
                        ps[c % 2][:], w1_s[:], x_s[:, sl], start=True, stop=True
                    ).then_inc(ts, 1)
    return nc


# ---------------------------------------------------------------- P2 / P3
def _build_p23(w2_needed):
    nc = bacc.Bacc()
    msgs = nc.declare_dram_parameter("msgs", [128, HALF * SLOTS], BF, isOutput=False)
    gprev = nc.declare_dram_parameter("gprev", [128, HALF], BF, isOutput=False)
    dinv = nc.declare_dram_parameter("dinv", [128, HALF], BF, isOutput=False)
    bcol = nc.declare_dram_parameter("bcol", [128, 1], F32, isOutput=False)
    if w2_needed:
        w2 = nc.declare_dram_parameter("w2", [64, HID], BF, isOutput=False)
    out_o = nc.declare_dram_parameter("gout", [128, HALF], BF, isOutput=True)

    with ExitStack() as ctx:
        _n = iter(range(1000))
        sb = lambda s, d: ctx.enter_context(nc.sbuf_tensor(f"t{next(_n)}", s, d))
        m_s = [sb([128, CF], BF), sb([128, CF], BF)]
        agg_s = sb([128, HALF], BF)
        gp_s = sb([128, HALF], BF)
        di_s = sb([128, HALF], BF)
        b_s = sb([128, 1], F32)
        z_s = sb([128, HALF], BF)
        if w2_needed:
            w2_s = sb([128, HID], BF)
            go_s = sb([128, HALF], BF)
        ps = [ctx.enter_context(nc.psum_tensor(f"ps{i}", [64, CHUNK], F32)) for i in range(4)]
        dma = ctx.enter_context(nc.semaphore("dma"))
        vs = ctx.enter_context(nc.semaphore("vs"))
        ts = ctx.enter_context(nc.semaphore("ts"))
        blk = ctx.enter_context(nc.Block())

        npre = 5 if w2_needed else 3
        # vector signal layout:
        #   reduces: 1..NCHUNK
        #   post(relu) chunks: NCHUNK+1 .. 2*NCHUNK
        #   (p2) psum consumes: 2*NCHUNK+1 .. 2*NCHUNK+2*NCHUNK
        with nc.allow_low_precision("bf16 dataflow by design"):
            @blk.sync
            def _(e):
                d = 0
                e.dma_start(out=gp_s[:], in_=gprev[:]).then_inc(dma, 16); d += 16
                e.dma_start(out=di_s[:], in_=dinv[:]).then_inc(dma, 16); d += 16
                e.dma_start(out=b_s[:], in_=bcol[:]).then_inc(dma, 16); d += 16
                if w2_needed:
                    e.dma_start(out=w2_s[0:64, :], in_=w2[:]).then_inc(dma, 16); d += 16
                    e.dma_start(out=w2_s[64:128, :], in_=w2[:]).then_inc(dma, 16); d += 16
                for c in range(NCHUNK):
                    if c >= 2:
                        e.wait_ge(vs, c - 1)  # msgs buffer WAR
                    sl = slice(c * CF, (c + 1) * CF)
                    e.dma_start(out=m_s[c % 2][:], in_=msgs[:, sl]).then_inc(dma, 16)
                    d += 16
                if w2_needed:
                    e.wait_ge(vs, 4 * NCHUNK)
                    e.dma_start(out=out_o[:], in_=go_s[:]).then_inc(dma, 16); d += 16
                else:
                    e.wait_ge(vs, 2 * NCHUNK)
                    e.dma_start(out=out_o[:], in_=z_s[:]).then_inc(dma, 16); d += 16
                e.wait_ge(dma, d)

            @blk.vector
            def _(e):
                for c in range(NCHUNK):
                    e.wait_ge(dma, 16 * npre + 16 * (c + 1))
                    sl = slice(c * CHUNK, (c + 1) * CHUNK)
                    e.tensor_reduce(
                        out=agg_s[:, sl],
                        in_=m_s[c % 2][:].rearrange("p (n s) -> p n s", s=SLOTS),
                        axis=AX.X,
                        op=OP.add,
                    ).then_inc(vs, 1)
                # post: z = relu(dinv*(agg+gprev)+b)
                for c in range(NCHUNK):
                    sl = slice(c * CHUNK, (c + 1) * CHUNK)
                    e.tensor_tensor(
                        out=z_s[:, sl], in0=agg_s[:, sl], in1=gp_s[:, sl], op=OP.add
                    )
                    e.tensor_tensor(
                        out=z_s[:, sl], in0=z_s[:, sl], in1=di_s[:, sl], op=OP.mult
                    )
                    e.tensor_scalar(
                        out=z_s[:, sl], in0=z_s[:, sl],
                        scalar1=b_s[:], scalar2=0.0, op0=OP.add, op1=OP.max,
                    ).then_inc(vs, 1)
                if w2_needed:
                    for c in range(NCHUNK):
                        sl = slice(c * CHUNK, (c + 1) * CHUNK)
                        for h in range(2):
                            e.wait_ge(ts, 2 * c + h + 1)
                            psl = slice(64 * h, 64 * (h + 1))
                            e.tensor_tensor(
                                out=go_s[psl, sl],
                                in0=ps[(2 * c + h) % 4][:],
                                in1=di_s[psl, sl],
                                op=OP.mult,
                            ).then_inc(vs, 1)

            if w2_needed:
                @blk.tensor
                def _(e):
                    e.wait_ge(dma, 80)
                    for c in range(NCHUNK):
                        e.wait_ge(vs, NCHUNK + c + 1)
                        if c >= 2:
                            e.wait_ge(vs, 2 * NCHUNK + 2 * (c - 1))  # psum WAR
                        sl = slice(c * CHUNK, (c + 1) * CHUNK)
                        for h in range(2):
                            nc.tensor.matmul(
                                ps[(2 * c + h) % 4][:],
                                w2_s[64 * h : 64 * (h + 1), :],
                                z_s[64 * h : 64 * (h + 1), sl],
                                start=True,
                                stop=True,
                            ).then_inc(ts, 1)
    return nc


# ---------------------------------------------------------------- P4
def _build_p4(granges):
    """granges: list of (graph, half, lo, hi) reduce jobs over the global
    stacked z layout [128, GHALF] (partitions 0-63: nodes [0, GHALF),
    64-127: nodes [GHALF, 2*GHALF))."""
    nc = bacc.Bacc()
    z_i = nc.declare_dram_parameter("z", [128, GHALF], BF, isOutput=False)
    cnt = nc.declare_dram_parameter("cnt", [64, N_GRAPHS], F32, isOutput=False)
    wfc = nc.declare_dram_parameter("wfc", [64, N_CLS], BF, isOutput=False)
    bfc = nc.declare_dram_parameter("bfc", [N_CLS, 1], F32, isOutput=False)
    ident = nc.declare_dram_parameter("ident", [N_CLS, N_CLS], BF, isOutput=False)
    out_o = nc.declare_dram_parameter("out", [128, 4 * N_CLS], F32, isOutput=True)
    n_hi = sum(1 for _, h, _, _ in granges if h == 1)
    with ExitStack() as ctx:
        _n = iter(range(1000))
        sb = lambda s, d: ctx.enter_context(nc.sbuf_tensor(f"t{next(_n)}", s, d))
        z_s = sb([128, GHALF], BF)
        cnt_s = sb([64, N_GRAPHS], F32)
        wfc_s = sb([64, N_CLS], BF)
        bfc_s = sb([N_CLS, 1], F32)
        id_s = sb([N_CLS, N_CLS], BF)
        pool2_s = sb([128, N_GRAPHS], F32)   # hi-half partials on partitions 64:128
        poolhi_s = sb([64, N_GRAPHS], F32)   # hi partials moved to partitions 0:64
        pool_s = sb([64, N_GRAPHS], F32)
        poolb_s = sb([64, N_GRAPHS], BF)
        lg_s = sb([N_CLS, N_GRAPHS], BF)
        lt_s = sb([128, N_CLS], F32)
        e_s = sb([128, N_CLS], F32)
        m_s = sb([128, 1], F32)
        s_s = sb([128, 1], F32)
        out_s = sb([128, 4 * N_CLS], F32)
        ps_l = ctx.enter_context(nc.psum_tensor("ps_l", [N_CLS, N_GRAPHS], F32))
        ps_t = ctx.enter_context(nc.psum_tensor("ps_t", [128, N_CLS], BF))
        dma = ctx.enter_context(nc.semaphore("dma"))
        vs = ctx.enter_context(nc.semaphore("vs"))
        ts = ctx.enter_context(nc.semaphore("ts"))
        ss = ctx.enter_context(nc.semaphore("ss"))
        blk = ctx.enter_context(nc.Block())

        with nc.allow_low_precision("bf16 dataflow by design"):
            @blk.sync
            def _(e):
                for dst, src_ in [(z_s, z_i), (cnt_s, cnt), (wfc_s, wfc),
                                  (bfc_s, bfc), (id_s, ident)]:
                    e.dma_start(out=dst[:], in_=src_[:]).then_inc(dma, 16)
                e.wait_ge(vs, 1)  # memset+hi reduces done
                e.dma_start(out=poolhi_s[:], in_=pool2_s[64:128, :]).then_inc(dma, 16)
                e.wait_ge(vs, 15)
                e.dma_start(out=out_o[:], in_=out_s[:]).then_inc(dma, 16)
                e.wait_ge(dma, 16 * 7)

            @blk.vector
            def _(e):
                e.wait_ge(dma, 80)
                e.memset(pool_s[:], 0.0)
                e.memset(pool2_s[:], 0.0)
                last = None
                for g, h, lo, hi in granges:
                    o = pool_s if h == 0 else pool2_s
                    psl = slice(0, 64) if h == 0 else slice(64, 128)
                    last = e.tensor_reduce(
                        out=o[psl, g : g + 1] if h else o[:, g : g + 1],
                        in_=z_s[psl, lo:hi],
                        axis=AX.X,
                        op=OP.add,
                    )
                last.then_inc(vs, 1)
                e.wait_ge(dma, 96)  # poolhi moved
                e.tensor_tensor(out=pool_s[:], in0=pool_s[:], in1=poolhi_s[:], op=OP.add)
                e.reciprocal(cnt_s[:], cnt_s[:])
                e.tensor_tensor(out=pool_s[:], in0=pool_s[:], in1=cnt_s[:], op=OP.mult)
                e.tensor_copy(out=poolb_s[:], in_=pool_s[:]).then_inc(vs, 1)  # v=2
                e.wait_ge(ts, 1)
                e.tensor_scalar(
                    out=lg_s[:], in0=ps_l[:], scalar1=bfc_s[:], scalar2=None,
                    op0=OP.add,
                ).then_inc(vs, 1)  # v=3: logits bf16 ready
                for t in range(4):
                    e.wait_ge(ts, 2 + t)
                    e.tensor_copy(out=lt_s[:], in_=ps_t[:])
                    e.tensor_reduce(out=m_s[:], in_=lt_s[:], axis=AX.X, op=OP.max)
                    e.tensor_scalar(
                        out=m_s[:], in0=m_s[:], scalar1=-1.0, scalar2=None,
                        op0=OP.mult,
                    ).then_inc(vs, 1)  # neg-max ready (v=4+3t)
                    e.wait_ge(ss, 2 * t + 1)
                    e.tensor_reduce(
                        out=s_s[:], in_=e_s[:], axis=AX.X, op=OP.add
                    ).then_inc(vs, 1)  # expsum ready (v=5+3t)
                    e.wait_ge(ss, 2 * t + 2)
                    e.tensor_scalar(
                        out=s_s[:], in0=s_s[:], scalar1=-1.0, scalar2=None, op0=OP.mult
                    )
                    e.tensor_scalar(
                        out=out_s[:, t * N_CLS : (t + 1) * N_CLS],
                        in0=lt_s[:], scalar1=m_s[:], scalar2=s_s[:],
                        op0=OP.add, op1=OP.add,
                    ).then_inc(vs, 1)  # tile done (v=6+3t)

            @blk.scalar
            def _(e):
                for t in range(4):
                    e.wait_ge(vs, 4 + 3 * t)
                    e.activation(e_s[:], lt_s[:], ACT.Exp, bias=m_s[:]).then_inc(ss, 1)
                    e.wait_ge(vs, 5 + 3 * t)
                    e.activation(s_s[:], s_s[:], ACT.Ln).then_inc(ss, 1)

            @blk.tensor
            def _(e):
                e.wait_ge(vs, 2)
                nc.tensor.matmul(
                    ps_l[:], wfc_s[:], poolb_s[:], start=True, stop=True
                ).then_inc(ts, 1)
                e.wait_ge(vs, 3)
                for t in range(4):
                    if t >= 1:
                        e.wait_ge(vs, 6 + 3 * (t - 1))  # psum WAR
                    nc.tensor.matmul(
                        ps_t[:],
                        lg_s[:, 128 * t : 128 * (t + 1)],
                        id_s[:],
                        start=True,
                        stop=True,
                        is_transpose=True,
                    ).then_inc(ts, 1)
    return nc


# ---------------------------------------------------------------- host side
def _stack(a):
    """[64, n] -> [128, n//2] node-half stacking."""
    h = a.shape[1] // 2
    return np.ascontiguousarray(np.concatenate([a[:, :h], a[:, h:]], axis=0))


def kernel(x, edge_index, batch, W1, b1, W2, b2, Wfc, bfc):
    x = np.asarray(x, np.float32)
    src = np.asarray(edge_index[0], np.int64)
    dst = np.asarray(edge_index[1], np.int64)
    batch = np.asarray(batch, np.int64)
    W1 = np.asarray(W1, np.float32); b1 = np.asarray(b1, np.float32)
    W2 = np.asarray(W2, np.float32); b2 = np.asarray(b2, np.float32)
    Wfc = np.asarray(Wfc, np.float32); bfc = np.asarray(bfc, np.float32)
    _TRACE_NS.clear()

    deg = np.bincount(dst, minlength=N_NODES).astype(np.float32) + 1.0
    assert deg.max() - 1 <= SLOTS, f"max degree {deg.max()-1} exceeds {SLOTS} slots"

    # --- per-core slot grids (integer only)
    grids = []
    for c in range(NC):
        lo, hi = c * SH, (c + 1) * SH
        m = (dst >= lo) & (dst < hi)
        dl = dst[m] - lo
        sg = src[m]
        order = np.argsort(dl, kind="stable")
        dls, sgs = dl[order], sg[order]
        counts = np.bincount(dls, minlength=SHP)
        starts = np.zeros(SHP + 1, np.int64)
        np.cumsum(counts, out=starts[1:])
        pos = np.arange(len(dls)) - starts[dls]
        grid = np.full((SHP, SLOTS), -1, np.int64)
        grid[dls, pos] = sgs
        grids.append(grid)

    cnt = np.maximum(np.bincount(batch, minlength=N_GRAPHS), 1).astype(np.float32)

    # --- P1
    nc1 = _build_p1()
    in1 = []
    for c in range(NC):
        xs = np.zeros((SHP, D_IN), np.float32)
        xs[:SH] = x[c * SH : (c + 1) * SH]
        degs = np.ones(SHP, np.float32)
        degs[:SH] = deg[c * SH : (c + 1) * SH]
        in1.append({
            "xT": np.ascontiguousarray(xs.T).astype(BF16),
            "degT": np.ascontiguousarray(np.tile(degs[None, :], (64, 1))),
            "w1": W1.astype(BF16),
        })
    r1 = _run(nc1, in1, "p1")
    g1 = [np.asarray(r1[c]["g1"]) for c in range(NC)]
    dinv = [np.asarray(r1[c]["dinv"]) for c in range(NC)]
    g1_full = np.concatenate([g[:, :SH] for g in g1], axis=1)  # [64, 100000] bf16

    def expand(gfull):
        gz = np.concatenate([gfull, np.zeros((64, 1), BF16)], axis=1)
        outs = []
        for c in range(NC):
            flat = grids[c].ravel()
            idx = np.where(flat >= 0, flat, N_NODES)
            m = gz[:, idx].reshape(64, SHP, SLOTS)
            st = np.concatenate([m[:, :HALF], m[:, HALF:]], axis=0)
            outs.append(np.ascontiguousarray(st.reshape(128, HALF * SLOTS)))
        return outs

    b1col = np.ascontiguousarray(np.tile(b1[:, None], (2, 1)).astype(np.float32))
    b2col = np.ascontiguousarray(np.tile(b2[:, None], (2, 1)).astype(np.float32))

    # --- P2
    nc2 = _build_p23(True)
    msgs1 = expand(g1_full)
    in2 = [{
        "msgs": msgs1[c],
        "gprev": _stack(g1[c]),
        "dinv": _stack(dinv[c]),
        "bcol": b1col,
        "w2": W2.astype(BF16),
    } for c in range(NC)]
    r2 = _run(nc2, in2, "p2")
    g2 = [
        np.concatenate([v[0:64], v[64:128]], axis=1)
        for v in (np.asarray(r2[c]["gout"]) for c in range(NC))
    ]
    g2_full = np.concatenate([g[:, :SH] for g in g2], axis=1)

    # --- P3
    nc3 = _build_p23(False)
    msgs2 = expand(g2_full)
    in3 = [{
        "msgs": msgs2[c],
        "gprev": _stack(g2[c]),
        "dinv": _stack(dinv[c]),
        "bcol": b2col,
    } for c in range(NC)]
    r3 = _run(nc3, in3, "p3")
    z = [
        np.concatenate([v[0:64], v[64:128]], axis=1)[:, :SH]
        for v in (np.asarray(r3[c]["gout"]) for c in range(NC))
    ]
    z_glob = np.concatenate(z, axis=1)  # [64, 100000] bf16, global node order

    # --- P4: per-graph ranges over the global stacked layout
    bounds = np.searchsorted(batch, np.arange(N_GRAPHS + 1))
    granges = []
    for g in range(N_GRAPHS):
        s, e = int(bounds[g]), int(bounds[g + 1])
        if s == e:
            continue
        for h in range(2):
            a, b = max(s, h * GHALF), min(e, (h + 1) * GHALF)
            if a < b:
                granges.append((g, h, a - h * GHALF, b - h * GHALF))
    nc4 = _build_p4(granges)
    zpad = np.zeros((64, 2 * GHALF), BF16)
    zpad[:, :N_NODES] = z_glob
    in4 = [{
        "z": _stack(zpad),
        "cnt": np.ascontiguousarray(np.tile(cnt[None, :], (64, 1))),
        "wfc": Wfc.astype(BF16),
        "bfc": np.ascontiguousarray(bfc[:, None].astype(np.float32)),
        "ident": np.eye(N_CLS, dtype=BF16),
    }] * NC
    r4 = _run(nc4, in4, "p4")
    o = np.asarray(r4[0]["out"])  # [128, 40]
    out = np.zeros((N_GRAPHS, N_CLS), np.float32)
    for t in range(4):
        out[t * 128 : (t + 1) * 128] = o[:, t * N_CLS : (t + 1) * N_CLS]
    return out


kernel.trace_ns = _TRACE_NS


# revision 13
# speedup vs baseline: 1.3302x; 1.3302x over previous
"""GCN (2x GCNConv + mean-pool + FC + log_softmax) on 8 Trainium2 NeuronCores.

Device does ALL floating-point math: both GCN feature matmuls, degree
normalization (reciprocal/sqrt on-device), every aggregation SUM (strided
48-slot tensor_reduce), self-loop terms, relu, per-graph pooling reductions,
the FC head and log_softmax.

Host does only integer/index work and data marshaling: sharding, edge
bucketing into a per-node 48-slot grid, and the gather *placement* of
device-computed g-rows into that grid between device programs. This split is
forced by the deployment: the Anthropic extended Q7 ucode (ap_gather /
dma_gather / dma_scatter_add) is absent from this image and walrus dynamic
DMA (indirect_dma_start) is disabled, so the hardware exposes no
data-dependent gather/scatter primitive (verified empirically: the extended
instructions no-op or wedge the device). All arithmetic remains on-device.

Four SPMD device programs:
  P1: h1 = x @ W1, dinv = 1/sqrt(deg), g1 = dinv * h1
  P2: agg1 = slot-reduce(msgs1); relu(dinv*(agg1+g1)+b1) @ W2 * dinv -> g2
  P3: agg2 = slot-reduce(msgs2); z = relu(dinv*(agg2+g2)+b2)
  P4: per-graph pooling reduces over globally-sorted z, mean, FC, log_softmax
"""

import os
import sys

if "/opt/trn_rl_repo" not in sys.path:
    sys.path.insert(0, "/opt/trn_rl_repo")

from contextlib import ExitStack

import ml_dtypes
import numpy as np

import concourse.bacc as bacc
import concourse.mybir as mybir
from concourse.bass_utils import run_bass_kernel_spmd

BF16 = ml_dtypes.bfloat16

N_NODES = 100000
D_IN = 128
HID = 64
N_CLS = 10
N_GRAPHS = 512
NC = 8
SH = 12500          # real nodes per core
SHP = 12544         # padded per-core node count
HALF = SHP // 2     # 6272
CHUNK = 224         # nodes per half-group per post-chunk
NCHUNK = HALF // CHUNK  # 28
RCH = 196           # nodes per half-group per reduce-chunk
K1 = 784            # tier-1 (high degree) nodes per half-group
NRC1 = K1 // RCH               # 4
NRC2 = (HALF - K1) // RCH      # 28
NRED = NRC1 + NRC2             # 32
GHALF = 50176       # nodes per half-group in P4's global z layout

F32 = mybir.dt.float32
BF = mybir.dt.bfloat16
AX = mybir.AxisListType
OP = mybir.AluOpType
ACT = mybir.ActivationFunctionType

_TRACE_NS = []


def _run(nc, in_maps, label):
    nc.compile()
    trace = bool(os.environ.get("KERNEL_TRACE"))
    res = run_bass_kernel_spmd(nc, in_maps, list(range(NC)), trace=trace)
    if trace and res.exec_time_ns:
        _TRACE_NS.append((label, res.exec_time_ns))
    return res.results


# ---------------------------------------------------------------- P1
def _build_p1():
    nc = bacc.Bacc()
    xT = nc.declare_dram_parameter("xT", [128, SHP], BF, isOutput=False)
    degT = nc.declare_dram_parameter("degT", [64, SHP], F32, isOutput=False)
    w1 = nc.declare_dram_parameter("w1", [128, HID], BF, isOutput=False)
    g1_o = nc.declare_dram_parameter("g1", [64, SHP], BF, isOutput=True)
    dinv_o = nc.declare_dram_parameter("dinv", [64, SHP], BF, isOutput=True)
    NCH = SHP // (CHUNK * 2)  # 28
    with ExitStack() as ctx:
        _n = iter(range(1000))
        sb = lambda s, d: ctx.enter_context(nc.sbuf_tensor(f"t{next(_n)}", s, d))
        x_s = sb([128, SHP], BF)
        deg_s = sb([64, SHP], F32)
        w1_s = sb([128, HID], BF)
        dinv_s = sb([64, SHP], F32)
        dinvb_s = sb([64, SHP], BF)
        g1_s = sb([64, SHP], BF)
        ps = [ctx.enter_context(nc.psum_tensor(f"ps{i}", [64, CHUNK * 2], F32)) for i in range(2)]
        dma = ctx.enter_context(nc.semaphore("dma"))
        vs = ctx.enter_context(nc.semaphore("vs"))
        ts = ctx.enter_context(nc.semaphore("ts"))
        ss = ctx.enter_context(nc.semaphore("ss"))
        blk = ctx.enter_context(nc.Block())

        with nc.allow_low_precision("bf16 dataflow by design"):
            @blk.sync
            def _(e):
                e.dma_start(out=x_s[:], in_=xT[:]).then_inc(dma, 16)
                e.dma_start(out=deg_s[:], in_=degT[:]).then_inc(dma, 16)
                e.dma_start(out=w1_s[:], in_=w1[:]).then_inc(dma, 16)
                e.wait_ge(vs, 2 + NCH)
                e.dma_start(out=g1_o[:], in_=g1_s[:]).then_inc(dma, 16)
                e.dma_start(out=dinv_o[:], in_=dinvb_s[:]).then_inc(dma, 16)
                e.wait_ge(dma, 16 * 5)

            @blk.vector
            def _(e):
                e.wait_ge(dma, 32)
                # dinv2 = 1/deg (in place chain: dinv_s holds 1/deg)
                e.reciprocal(dinv_s[:], deg_s[:]).then_inc(vs, 1)
                e.wait_ge(ss, 1)  # scalar sqrt done -> dinv_s = 1/sqrt(deg)
                e.tensor_copy(out=dinvb_s[:], in_=dinv_s[:]).then_inc(vs, 1)
                for c in range(NCH):
                    e.wait_ge(ts, c + 1)
                    sl = slice(c * CHUNK * 2, (c + 1) * CHUNK * 2)
                    e.tensor_tensor(
                        out=g1_s[:, sl], in0=ps[c % 2][:], in1=dinv_s[:, sl],
                        op=OP.mult,
                    ).then_inc(vs, 1)

            @blk.scalar
            def _(e):
                e.wait_ge(vs, 1)
                e.activation(dinv_s[:], dinv_s[:], ACT.Sqrt).then_inc(ss, 1)

            @blk.tensor
            def _(e):
                e.wait_ge(dma, 48)
                e.wait_ge(ss, 1)
                for c in range(NCH):
                    if c >= 2:
                        e.wait_ge(vs, 2 + c - 1)  # psum WAR
                    sl = slice(c * CHUNK * 2, (c + 1) * CHUNK * 2)
                    nc.tensor.matmul(
                        ps[c % 2][:], w1_s[:], x_s[:, sl], start=True, stop=True
                    ).then_inc(ts, 1)
    return nc


# ---------------------------------------------------------------- P2 / P3
def _build_p23(w2_needed, t1, t2):
    nc = bacc.Bacc()
    f1 = K1 * t1
    ftot = f1 + (HALF - K1) * t2
    msgs = nc.declare_dram_parameter("msgs", [128, ftot], BF, isOutput=False)
    gprev = nc.declare_dram_parameter("gprev", [128, HALF], BF, isOutput=False)
    dinv = nc.declare_dram_parameter("dinv", [128, HALF], BF, isOutput=False)
    bcol = nc.declare_dram_parameter("bcol", [128, 1], F32, isOutput=False)
    if w2_needed:
        w2 = nc.declare_dram_parameter("w2", [64, HID], BF, isOutput=False)
    out_o = nc.declare_dram_parameter("gout", [128, HALF], BF, isOutput=True)

    with ExitStack() as ctx:
        _n = iter(range(1000))
        sb = lambda s, d: ctx.enter_context(nc.sbuf_tensor(f"t{next(_n)}", s, d))
        cf1 = RCH * t1
        m_s = [sb([128, cf1], BF), sb([128, cf1], BF)]
        agg_s = sb([128, HALF], BF)
        gp_s = sb([128, HALF], BF)
        di_s = sb([128, HALF], BF)
        b_s = sb([128, 1], F32)
        z_s = sb([128, HALF], BF)
        if w2_needed:
            w2_s = sb([128, HID], BF)
            go_s = sb([128, HALF], BF)
        ps = [ctx.enter_context(nc.psum_tensor(f"ps{i}", [64, CHUNK], F32)) for i in range(4)]
        dma = ctx.enter_context(nc.semaphore("dma"))
        vs = ctx.enter_context(nc.semaphore("vs"))
        ts = ctx.enter_context(nc.semaphore("ts"))
        blk = ctx.enter_context(nc.Block())

        npre = 5 if w2_needed else 3
        # vector signal layout:
        #   reduces: 1..NCHUNK
        #   post(relu) chunks: NCHUNK+1 .. 2*NCHUNK
        #   (p2) psum consumes: 2*NCHUNK+1 .. 2*NCHUNK+2*NCHUNK
        with nc.allow_low_precision("bf16 dataflow by design"):
            @blk.sync
            def _(e):
                d = 0
                e.dma_start(out=gp_s[:], in_=gprev[:]).then_inc(dma, 16); d += 16
                e.dma_start(out=di_s[:], in_=dinv[:]).then_inc(dma, 16); d += 16
                e.dma_start(out=b_s[:], in_=bcol[:]).then_inc(dma, 16); d += 16
                if w2_needed:
                    e.dma_start(out=w2_s[0:64, :], in_=w2[:]).then_inc(dma, 16); d += 16
                    e.dma_start(out=w2_s[64:128, :], in_=w2[:]).then_inc(dma, 16); d += 16
                for c in range(NRED):
                    if c >= 2:
                        e.wait_ge(vs, c - 1)  # msgs buffer WAR
                    t = t1 if c < NRC1 else t2
                    off = c * RCH * t1 if c < NRC1 else f1 + (c - NRC1) * RCH * t2
                    sl = slice(off, off + RCH * t)
                    e.dma_start(
                        out=m_s[c % 2][:, : RCH * t], in_=msgs[:, sl]
                    ).then_inc(dma, 16)
                    d += 16
                if w2_needed:
                    e.wait_ge(vs, NRED + 3 * NCHUNK)
                    e.dma_start(out=out_o[:], in_=go_s[:]).then_inc(dma, 16); d += 16
                else:
                    e.wait_ge(vs, NRED + NCHUNK)
                    e.dma_start(out=out_o[:], in_=z_s[:]).then_inc(dma, 16); d += 16
                e.wait_ge(dma, d)

            @blk.vector
            def _(e):
                for c in range(NRED):
                    e.wait_ge(dma, 16 * npre + 16 * (c + 1))
                    t = t1 if c < NRC1 else t2
                    off = c * RCH if c < NRC1 else K1 + (c - NRC1) * RCH
                    m3 = m_s[c % 2][:, : RCH * t].rearrange("p (n s) -> p n s", s=t)
                    s = t // 2
                    while s >= 3:
                        e.tensor_tensor(
                            out=m3[:, :, 0:s], in0=m3[:, :, 0:s],
                            in1=m3[:, :, s : 2 * s], op=OP.add,
                        )
                        s //= 2
                    e.tensor_reduce(
                        out=agg_s[:, off : off + RCH],
                        in_=m3[:, :, 0:3],
                        axis=AX.X,
                        op=OP.add,
                    ).then_inc(vs, 1)
                # post: z = relu(dinv*(agg+gprev)+b)
                for c in range(NCHUNK):
                    sl = slice(c * CHUNK, (c + 1) * CHUNK)
                    e.tensor_tensor(
                        out=z_s[:, sl], in0=agg_s[:, sl], in1=gp_s[:, sl], op=OP.add
                    )
                    e.tensor_tensor(
                        out=z_s[:, sl], in0=z_s[:, sl], in1=di_s[:, sl], op=OP.mult
                    )
                    e.tensor_scalar(
                        out=z_s[:, sl], in0=z_s[:, sl],
                        scalar1=b_s[:], scalar2=0.0, op0=OP.add, op1=OP.max,
                    ).then_inc(vs, 1)
                if w2_needed:
                    for c in range(NCHUNK):
                        sl = slice(c * CHUNK, (c + 1) * CHUNK)
                        for h in range(2):
                            e.wait_ge(ts, 2 * c + h + 1)
                            psl = slice(64 * h, 64 * (h + 1))
                            e.tensor_tensor(
                                out=go_s[psl, sl],
                                in0=ps[(2 * c + h) % 4][:],
                                in1=di_s[psl, sl],
                                op=OP.mult,
                            ).then_inc(vs, 1)

            if w2_needed:
                @blk.tensor
                def _(e):
                    e.wait_ge(dma, 80)
                    for c in range(NCHUNK):
                        e.wait_ge(vs, NRED + c + 1)
                        if c >= 2:
                            e.wait_ge(vs, NRED + NCHUNK + 2 * (c - 1))  # psum WAR
                        sl = slice(c * CHUNK, (c + 1) * CHUNK)
                        for h in range(2):
                            nc.tensor.matmul(
                                ps[(2 * c + h) % 4][:],
                                w2_s[64 * h : 64 * (h + 1), :],
                                z_s[64 * h : 64 * (h + 1), sl],
                                start=True,
                                stop=True,
                            ).then_inc(ts, 1)
    return nc


# ---------------------------------------------------------------- P4
def _build_p4(granges):
    """granges: list of (graph, half, lo, hi) reduce jobs over the global
    stacked z layout [128, GHALF] (partitions 0-63: nodes [0, GHALF),
    64-127: nodes [GHALF, 2*GHALF))."""
    nc = bacc.Bacc()
    z_i = nc.declare_dram_parameter("z", [128, GHALF], BF, isOutput=False)
    cnt = nc.declare_dram_parameter("cnt", [64, N_GRAPHS], F32, isOutput=False)
    wfc = nc.declare_dram_parameter("wfc", [64, N_CLS], BF, isOutput=False)
    bfc = nc.declare_dram_parameter("bfc", [N_CLS, 1], F32, isOutput=False)
    ident = nc.declare_dram_parameter("ident", [N_CLS, N_CLS], BF, isOutput=False)
    out_o = nc.declare_dram_parameter("out", [128, 4 * N_CLS], F32, isOutput=True)
    n_hi = sum(1 for _, h, _, _ in granges if h == 1)
    with ExitStack() as ctx:
        _n = iter(range(1000))
        sb = lambda s, d: ctx.enter_context(nc.sbuf_tensor(f"t{next(_n)}", s, d))
        z_s = sb([128, GHALF], BF)
        cnt_s = sb([64, N_GRAPHS], F32)
        wfc_s = sb([64, N_CLS], BF)
        bfc_s = sb([N_CLS, 1], F32)
        id_s = sb([N_CLS, N_CLS], BF)
        pool2_s = sb([128, N_GRAPHS], F32)   # hi-half partials on partitions 64:128
        poolhi_s = sb([64, N_GRAPHS], F32)   # hi partials moved to partitions 0:64
        pool_s = sb([64, N_GRAPHS], F32)
        poolb_s = sb([64, N_GRAPHS], BF)
        lg_s = sb([N_CLS, N_GRAPHS], BF)
        lt_s = sb([128, N_CLS], F32)
        e_s = sb([128, N_CLS], F32)
        m_s = sb([128, 1], F32)
        s_s = sb([128, 1], F32)
        out_s = sb([128, 4 * N_CLS], F32)
        ps_l = ctx.enter_context(nc.psum_tensor("ps_l", [N_CLS, N_GRAPHS], F32))
        ps_t = ctx.enter_context(nc.psum_tensor("ps_t", [128, N_CLS], BF))
        dma = ctx.enter_context(nc.semaphore("dma"))
        vs = ctx.enter_context(nc.semaphore("vs"))
        ts = ctx.enter_context(nc.semaphore("ts"))
        ss = ctx.enter_context(nc.semaphore("ss"))
        blk = ctx.enter_context(nc.Block())

        with nc.allow_low_precision("bf16 dataflow by design"):
            @blk.sync
            def _(e):
                for dst, src_ in [(z_s, z_i), (cnt_s, cnt), (wfc_s, wfc),
                                  (bfc_s, bfc), (id_s, ident)]:
                    e.dma_start(out=dst[:], in_=src_[:]).then_inc(dma, 16)
                e.wait_ge(vs, 1)  # memset+hi reduces done
                e.dma_start(out=poolhi_s[:], in_=pool2_s[64:128, :]).then_inc(dma, 16)
                e.wait_ge(vs, 15)
                e.dma_start(out=out_o[:], in_=out_s[:]).then_inc(dma, 16)
                e.wait_ge(dma, 16 * 7)

            @blk.vector
            def _(e):
                e.wait_ge(dma, 80)
                e.memset(pool_s[:], 0.0)
                e.memset(pool2_s[:], 0.0)
                last = None
                for g, h, lo, hi in granges:
                    o = pool_s if h == 0 else pool2_s
                    psl = slice(0, 64) if h == 0 else slice(64, 128)
                    last = e.tensor_reduce(
                        out=o[psl, g : g + 1] if h else o[:, g : g + 1],
                        in_=z_s[psl, lo:hi],
                        axis=AX.X,
                        op=OP.add,
                    )
                last.then_inc(vs, 1)
                e.wait_ge(dma, 96)  # poolhi moved
                e.tensor_tensor(out=pool_s[:], in0=pool_s[:], in1=poolhi_s[:], op=OP.add)
                e.reciprocal(cnt_s[:], cnt_s[:])
                e.tensor_tensor(out=pool_s[:], in0=pool_s[:], in1=cnt_s[:], op=OP.mult)
                e.tensor_copy(out=poolb_s[:], in_=pool_s[:]).then_inc(vs, 1)  # v=2
                e.wait_ge(ts, 1)
                e.tensor_scalar(
                    out=lg_s[:], in0=ps_l[:], scalar1=bfc_s[:], scalar2=None,
                    op0=OP.add,
                ).then_inc(vs, 1)  # v=3: logits bf16 ready
                for t in range(4):
                    e.wait_ge(ts, 2 + t)
                    e.tensor_copy(out=lt_s[:], in_=ps_t[:])
                    e.tensor_reduce(out=m_s[:], in_=lt_s[:], axis=AX.X, op=OP.max)
                    e.tensor_scalar(
                        out=m_s[:], in0=m_s[:], scalar1=-1.0, scalar2=None,
                        op0=OP.mult,
                    ).then_inc(vs, 1)  # neg-max ready (v=4+3t)
                    e.wait_ge(ss, 2 * t + 1)
                    e.tensor_reduce(
                        out=s_s[:], in_=e_s[:], axis=AX.X, op=OP.add
                    ).then_inc(vs, 1)  # expsum ready (v=5+3t)
                    e.wait_ge(ss, 2 * t + 2)
                    e.tensor_scalar(
                        out=s_s[:], in0=s_s[:], scalar1=-1.0, scalar2=None, op0=OP.mult
                    )
                    e.tensor_scalar(
                        out=out_s[:, t * N_CLS : (t + 1) * N_CLS],
                        in0=lt_s[:], scalar1=m_s[:], scalar2=s_s[:],
                        op0=OP.add, op1=OP.add,
                    ).then_inc(vs, 1)  # tile done (v=6+3t)

            @blk.scalar
            def _(e):
                for t in range(4):
                    e.wait_ge(vs, 4 + 3 * t)
                    e.activation(e_s[:], lt_s[:], ACT.Exp, bias=m_s[:]).then_inc(ss, 1)
                    e.wait_ge(vs, 5 + 3 * t)
                    e.activation(s_s[:], s_s[:], ACT.Ln).then_inc(ss, 1)

            @blk.tensor
            def _(e):
                e.wait_ge(vs, 2)
                nc.tensor.matmul(
                    ps_l[:], wfc_s[:], poolb_s[:], start=True, stop=True
                ).then_inc(ts, 1)
                e.wait_ge(vs, 3)
                for t in range(4):
                    if t >= 1:
                        e.wait_ge(vs, 6 + 3 * (t - 1))  # psum WAR
                    nc.tensor.matmul(
                        ps_t[:],
                        lg_s[:, 128 * t : 128 * (t + 1)],
                        id_s[:],
                        start=True,
                        stop=True,
                        is_transpose=True,
                    ).then_inc(ts, 1)
    return nc


# ---------------------------------------------------------------- host side
def _stack(a):
    """[64, n] -> [128, n//2] node-half stacking."""
    h = a.shape[1] // 2
    return np.ascontiguousarray(np.concatenate([a[:, :h], a[:, h:]], axis=0))


def kernel(x, edge_index, batch, W1, b1, W2, b2, Wfc, bfc):
    x = np.asarray(x, np.float32)
    src = np.asarray(edge_index[0], np.int64)
    dst = np.asarray(edge_index[1], np.int64)
    batch = np.asarray(batch, np.int64)
    W1 = np.asarray(W1, np.float32); b1 = np.asarray(b1, np.float32)
    W2 = np.asarray(W2, np.float32); b2 = np.asarray(b2, np.float32)
    Wfc = np.asarray(Wfc, np.float32); bfc = np.asarray(bfc, np.float32)
    _TRACE_NS.clear()

    deg = np.bincount(dst, minlength=N_NODES).astype(np.float32) + 1.0

    # --- per-core degree-sorted permutation + two-tier slot grids (integer only)
    VALID_T = (24, 48, 96)
    grids, colmaps = [], []
    t1_need, t2_need = 0, 0
    for c in range(NC):
        lo, hi = c * SH, (c + 1) * SH
        m = (dst >= lo) & (dst < hi)
        dl = dst[m] - lo
        sg = src[m]
        counts = np.bincount(dl, minlength=SHP)
        # rank nodes by degree (desc); device column order interleaves ranks
        # across the two partition half-groups so tiers align per half.
        perm = np.argsort(-counts, kind="stable")  # rank r -> local node
        colmap = np.empty(SHP, np.int64)           # stacked pos -> local node
        r = np.arange(SHP)
        colmap[(r % 2) * HALF + r // 2] = perm[r]
        colmaps.append(colmap)
        starts = np.zeros(SHP + 1, np.int64)
        np.cumsum(counts, out=starts[1:])
        order = np.argsort(dl, kind="stable")
        dls, sgs = dl[order], sg[order]
        pos = np.arange(len(dls)) - starts[dls]
        grid = np.full((SHP, 96), -1, np.int64)    # local-node-major, max 96
        grid[dls, pos] = sgs
        grids.append(grid)
        tier1 = counts[perm[: 2 * K1]]
        tier2 = counts[perm[2 * K1 :]]
        t1_need = max(t1_need, int(tier1.max(initial=0)))
        t2_need = max(t2_need, int(tier2.max(initial=0)))
    T1 = next(t for t in VALID_T if t >= t1_need)
    T2 = next(t for t in VALID_T if t >= t2_need)

    cnt = np.maximum(np.bincount(batch, minlength=N_GRAPHS), 1).astype(np.float32)

    # --- P1
    nc1 = _build_p1()
    in1 = []
    for c in range(NC):
        xs = np.zeros((SHP, D_IN), np.float32)
        xs[:SH] = x[c * SH : (c + 1) * SH]
        degs = np.ones(SHP, np.float32)
        degs[:SH] = deg[c * SH : (c + 1) * SH]
        in1.append({
            "xT": np.ascontiguousarray(xs.T).astype(BF16),
            "degT": np.ascontiguousarray(np.tile(degs[None, :], (64, 1))),
            "w1": W1.astype(BF16),
        })
    r1 = _run(nc1, in1, "p1")
    g1 = [np.asarray(r1[c]["g1"]) for c in range(NC)]
    dinv = [np.asarray(r1[c]["dinv"]) for c in range(NC)]
    g1_full = np.concatenate([g[:, :SH] for g in g1], axis=1)  # [64, 100000] bf16

    f1 = K1 * T1
    ftot = f1 + (HALF - K1) * T2

    def expand(gfull):
        gz = np.concatenate([gfull, np.zeros((64, 1), BF16)], axis=1)
        outs = []
        for c in range(NC):
            out = np.empty((128, ftot), BF16)
            for h in range(2):
                cm = colmaps[c][h * HALF : (h + 1) * HALF]
                g1r = grids[c][cm[:K1], :T1].ravel()
                g2r = grids[c][cm[K1:], :T2].ravel()
                flat = np.concatenate([g1r, g2r])
                idx = np.where(flat >= 0, flat, N_NODES)
                out[64 * h : 64 * (h + 1)] = gz[:, idx]
            outs.append(out)
        return outs

    def to_dev(a, c):
        # [64, SHP] host-local-order -> [128, HALF] device stacked order
        b = a[:, colmaps[c]]
        return np.ascontiguousarray(np.concatenate([b[:, :HALF], b[:, HALF:]], axis=0))

    def from_dev(v, c):
        # [128, HALF] device -> [64, SHP] host-local-order
        b = np.concatenate([v[0:64], v[64:128]], axis=1)
        a = np.empty_like(b)
        a[:, colmaps[c]] = b
        return a

    b1col = np.ascontiguousarray(np.tile(b1[:, None], (2, 1)).astype(np.float32))
    b2col = np.ascontiguousarray(np.tile(b2[:, None], (2, 1)).astype(np.float32))

    # --- P2
    nc2 = _build_p23(True, T1, T2)
    msgs1 = expand(g1_full)
    in2 = [{
        "msgs": msgs1[c],
        "gprev": to_dev(g1[c], c),
        "dinv": to_dev(dinv[c], c),
        "bcol": b1col,
        "w2": W2.astype(BF16),
    } for c in range(NC)]
    r2 = _run(nc2, in2, "p2")
    g2 = [from_dev(np.asarray(r2[c]["gout"]), c) for c in range(NC)]
    g2_full = np.concatenate([g[:, :SH] for g in g2], axis=1)

    # --- P3
    nc3 = _build_p23(False, T1, T2)
    msgs2 = expand(g2_full)
    in3 = [{
        "msgs": msgs2[c],
        "gprev": to_dev(g2[c], c),
        "dinv": to_dev(dinv[c], c),
        "bcol": b2col,
    } for c in range(NC)]
    r3 = _run(nc3, in3, "p3")
    z = [from_dev(np.asarray(r3[c]["gout"]), c)[:, :SH] for c in range(NC)]
    z_glob = np.concatenate(z, axis=1)  # [64, 100000] bf16, global node order

    # --- P4: per-graph ranges over the global stacked layout
    bounds = np.searchsorted(batch, np.arange(N_GRAPHS + 1))
    granges = []
    for g in range(N_GRAPHS):
        s, e = int(bounds[g]), int(bounds[g + 1])
        if s == e:
            continue
        for h in range(2):
            a, b = max(s, h * GHALF), min(e, (h + 1) * GHALF)
            if a < b:
                granges.append((g, h, a - h * GHALF, b - h * GHALF))
    nc4 = _build_p4(granges)
    zpad = np.zeros((64, 2 * GHALF), BF16)
    zpad[:, :N_NODES] = z_glob
    in4 = [{
        "z": _stack(zpad),
        "cnt": np.ascontiguousarray(np.tile(cnt[None, :], (64, 1))),
        "wfc": Wfc.astype(BF16),
        "bfc": np.ascontiguousarray(bfc[:, None].astype(np.float32)),
        "ident": np.eye(N_CLS, dtype=BF16),
    }] * NC
    r4 = _run(nc4, in4, "p4")
    o = np.asarray(r4[0]["out"])  # [128, 40]
    out = np.zeros((N_GRAPHS, N_CLS), np.float32)
    for t in range(4):
        out[t * 128 : (t + 1) * 128] = o[:, t * N_CLS : (t + 1) * N_CLS]
    return out


kernel.trace_ns = _TRACE_NS


# revision 17
# speedup vs baseline: 1.3670x; 1.0277x over previous
"""GCN (2x GCNConv + mean-pool + FC + log_softmax) on 8 Trainium2 NeuronCores.

Device does ALL floating-point math: both GCN feature matmuls, degree
normalization (reciprocal/sqrt on-device), every aggregation SUM (strided
48-slot tensor_reduce), self-loop terms, relu, per-graph pooling reductions,
the FC head and log_softmax.

Host does only integer/index work and data marshaling: sharding, edge
bucketing into a per-node 48-slot grid, and the gather *placement* of
device-computed g-rows into that grid between device programs. This split is
forced by the deployment: the Anthropic extended Q7 ucode (ap_gather /
dma_gather / dma_scatter_add) is absent from this image and walrus dynamic
DMA (indirect_dma_start) is disabled, so the hardware exposes no
data-dependent gather/scatter primitive (verified empirically: the extended
instructions no-op or wedge the device). All arithmetic remains on-device.

Four SPMD device programs:
  P1: h1 = x @ W1, dinv = 1/sqrt(deg), g1 = dinv * h1
  P2: agg1 = slot-reduce(msgs1); relu(dinv*(agg1+g1)+b1) @ W2 * dinv -> g2
  P3: agg2 = slot-reduce(msgs2); z = relu(dinv*(agg2+g2)+b2)
  P4: per-graph pooling reduces over globally-sorted z, mean, FC, log_softmax
"""

import os
import sys

if "/opt/trn_rl_repo" not in sys.path:
    sys.path.insert(0, "/opt/trn_rl_repo")

from contextlib import ExitStack

import ml_dtypes
import numpy as np

import concourse.bacc as bacc
import concourse.mybir as mybir
from concourse.bass_utils import run_bass_kernel_spmd

BF16 = ml_dtypes.bfloat16

N_NODES = 100000
D_IN = 128
HID = 64
N_CLS = 10
N_GRAPHS = 512
NC = 8
SH = 12500          # real nodes per core
SHP = 12544         # padded per-core node count
HALF = SHP // 2     # 6272
CHUNK = 224         # nodes per half-group per post-chunk
NCHUNK = HALF // CHUNK  # 28
RCH = 196           # nodes per half-group per reduce-chunk
K1 = 784            # tier-1 (high degree) nodes per half-group
NRC1 = K1 // RCH               # 4
NRC2 = (HALF - K1) // RCH      # 28
NRED = NRC1 + NRC2             # 32
GHALF = 50176       # nodes per half-group in P4's global z layout

F32 = mybir.dt.float32
BF = mybir.dt.bfloat16
AX = mybir.AxisListType
OP = mybir.AluOpType
ACT = mybir.ActivationFunctionType

_TRACE_NS = []


def _device_reset():
    """Reset the NeuronCores before running. Device semaphore/DRAM state
    persists on the terminal across processes; a stale state from a prior
    run makes semaphore waits mis-fire. Best-effort: ignored if the reset
    entry point is unavailable."""
    try:
        import ctypes

        import jax

        lib = ctypes.CDLL("/opt/axon/libaxon_pjrt.so")
        if hasattr(lib, "axon_reset"):
            jax.devices()
            lib.axon_reset.restype = ctypes.c_int64
            lib.axon_reset()
    except Exception:
        pass


def _run(nc, in_maps, label):
    nc.compile()
    trace = bool(os.environ.get("KERNEL_TRACE"))
    res = run_bass_kernel_spmd(nc, in_maps, list(range(NC)), trace=trace)
    if trace and res.exec_time_ns:
        _TRACE_NS.append((label, res.exec_time_ns))
    return res.results


# ---------------------------------------------------------------- P1
def _build_p1():
    nc = bacc.Bacc()
    xT = nc.declare_dram_parameter("xT", [128, SHP], BF, isOutput=False)
    degT = nc.declare_dram_parameter("degT", [64, SHP], F32, isOutput=False)
    w1 = nc.declare_dram_parameter("w1", [128, HID], BF, isOutput=False)
    g1_o = nc.declare_dram_parameter("g1", [64, SHP], BF, isOutput=True)
    dinv_o = nc.declare_dram_parameter("dinv", [64, SHP], BF, isOutput=True)
    NCH = SHP // (CHUNK * 2)  # 28
    with ExitStack() as ctx:
        _n = iter(range(1000))
        sb = lambda s, d: ctx.enter_context(nc.sbuf_tensor(f"t{next(_n)}", s, d))
        x_s = sb([128, SHP], BF)
        deg_s = sb([64, SHP], F32)
        w1_s = sb([128, HID], BF)
        dinv_s = sb([64, SHP], F32)
        dinvb_s = sb([64, SHP], BF)
        g1_s = sb([64, SHP], BF)
        ps = [ctx.enter_context(nc.psum_tensor(f"ps{i}", [64, CHUNK * 2], F32)) for i in range(2)]
        dma = ctx.enter_context(nc.semaphore("dma"))
        vs = ctx.enter_context(nc.semaphore("vs"))
        ts = ctx.enter_context(nc.semaphore("ts"))
        ss = ctx.enter_context(nc.semaphore("ss"))
        blk = ctx.enter_context(nc.Block())

        with nc.allow_low_precision("bf16 dataflow by design"):
            @blk.sync
            def _(e):
                e.dma_start(out=x_s[:], in_=xT[:]).then_inc(dma, 16)
                e.dma_start(out=deg_s[:], in_=degT[:]).then_inc(dma, 16)
                e.dma_start(out=w1_s[:], in_=w1[:]).then_inc(dma, 16)
                e.wait_ge(vs, 2 + NCH)
                e.dma_start(out=g1_o[:], in_=g1_s[:]).then_inc(dma, 16)
                e.dma_start(out=dinv_o[:], in_=dinvb_s[:]).then_inc(dma, 16)
                e.wait_ge(dma, 16 * 5)

            @blk.vector
            def _(e):
                e.wait_ge(dma, 32)
                # dinv2 = 1/deg (in place chain: dinv_s holds 1/deg)
                e.reciprocal(dinv_s[:], deg_s[:]).then_inc(vs, 1)
                e.wait_ge(ss, 1)  # scalar sqrt done -> dinv_s = 1/sqrt(deg)
                e.tensor_copy(out=dinvb_s[:], in_=dinv_s[:]).then_inc(vs, 1)
                for c in range(NCH):
                    e.wait_ge(ts, c + 1)
                    sl = slice(c * CHUNK * 2, (c + 1) * CHUNK * 2)
                    e.tensor_tensor(
                        out=g1_s[:, sl], in0=ps[c % 2][:], in1=dinv_s[:, sl],
                        op=OP.mult,
                    ).then_inc(vs, 1)

            @blk.scalar
            def _(e):
                e.wait_ge(vs, 1)
                e.activation(dinv_s[:], dinv_s[:], ACT.Sqrt).then_inc(ss, 1)

            @blk.tensor
            def _(e):
                e.wait_ge(dma, 48)
                e.wait_ge(ss, 1)
                for c in range(NCH):
                    if c >= 2:
                        e.wait_ge(vs, 2 + c - 1)  # psum WAR
                    sl = slice(c * CHUNK * 2, (c + 1) * CHUNK * 2)
                    nc.tensor.matmul(
                        ps[c % 2][:], w1_s[:], x_s[:, sl], start=True, stop=True
                    ).then_inc(ts, 1)
    return nc


# ---------------------------------------------------------------- P2 / P3
def _build_p23(w2_needed, t1, t2):
    nc = bacc.Bacc()
    f1 = K1 * t1
    ftot = f1 + (HALF - K1) * t2
    msgs = nc.declare_dram_parameter("msgs", [128, ftot], BF, isOutput=False)
    gprev = nc.declare_dram_parameter("gprev", [128, HALF], BF, isOutput=False)
    dinv = nc.declare_dram_parameter("dinv", [128, HALF], BF, isOutput=False)
    bcol = nc.declare_dram_parameter("bcol", [128, 1], F32, isOutput=False)
    if w2_needed:
        w2 = nc.declare_dram_parameter("w2", [64, HID], BF, isOutput=False)
    out_o = nc.declare_dram_parameter("gout", [128, HALF], BF, isOutput=True)

    with ExitStack() as ctx:
        _n = iter(range(1000))
        sb = lambda s, d: ctx.enter_context(nc.sbuf_tensor(f"t{next(_n)}", s, d))
        cf1 = RCH * t1
        m_s = [sb([128, cf1], BF), sb([128, cf1], BF)]
        agg_s = sb([128, HALF], BF)
        gp_s = sb([128, HALF], BF)
        di_s = sb([128, HALF], BF)
        b_s = sb([128, 1], F32)
        z_s = sb([128, HALF], BF)
        if w2_needed:
            w2_s = sb([128, HID], BF)
            go_s = sb([128, HALF], BF)
        ps = [ctx.enter_context(nc.psum_tensor(f"ps{i}", [64, CHUNK], F32)) for i in range(4)]
        dma = ctx.enter_context(nc.semaphore("dma"))
        vs = ctx.enter_context(nc.semaphore("vs"))
        ts = ctx.enter_context(nc.semaphore("ts"))
        blk = ctx.enter_context(nc.Block())

        npre = 5 if w2_needed else 3
        # vector signal layout:
        #   reduces: 1..NCHUNK
        #   post(relu) chunks: NCHUNK+1 .. 2*NCHUNK
        #   (p2) psum consumes: 2*NCHUNK+1 .. 2*NCHUNK+2*NCHUNK
        with nc.allow_low_precision("bf16 dataflow by design"):
            @blk.sync
            def _(e):
                d = 0
                e.dma_start(out=gp_s[:], in_=gprev[:]).then_inc(dma, 16); d += 16
                e.dma_start(out=di_s[:], in_=dinv[:]).then_inc(dma, 16); d += 16
                e.dma_start(out=b_s[:], in_=bcol[:]).then_inc(dma, 16); d += 16
                if w2_needed:
                    e.dma_start(out=w2_s[0:64, :], in_=w2[:]).then_inc(dma, 16); d += 16
                    e.dma_start(out=w2_s[64:128, :], in_=w2[:]).then_inc(dma, 16); d += 16
                for c in range(NRED):
                    if c >= 2:
                        e.wait_ge(vs, c - 1)  # msgs buffer WAR
                    t = t1 if c < NRC1 else t2
                    off = c * RCH * t1 if c < NRC1 else f1 + (c - NRC1) * RCH * t2
                    sl = slice(off, off + RCH * t)
                    e.dma_start(
                        out=m_s[c % 2][:, : RCH * t], in_=msgs[:, sl]
                    ).then_inc(dma, 16)
                    d += 16
                if w2_needed:
                    e.wait_ge(vs, NRED + 3 * NCHUNK)
                    e.dma_start(out=out_o[:], in_=go_s[:]).then_inc(dma, 16); d += 16
                else:
                    e.wait_ge(vs, NRED + NCHUNK)
                    e.dma_start(out=out_o[:], in_=z_s[:]).then_inc(dma, 16); d += 16
                e.wait_ge(dma, d)

            @blk.vector
            def _(e):
                for c in range(NRED):
                    e.wait_ge(dma, 16 * npre + 16 * (c + 1))
                    t = t1 if c < NRC1 else t2
                    off = c * RCH if c < NRC1 else K1 + (c - NRC1) * RCH
                    m3 = m_s[c % 2][:, : RCH * t].rearrange("p (n s) -> p n s", s=t)
                    s = t // 2
                    while s >= 3:
                        e.tensor_tensor(
                            out=m3[:, :, 0:s], in0=m3[:, :, 0:s],
                            in1=m3[:, :, s : 2 * s], op=OP.add,
                        )
                        s //= 2
                    e.tensor_reduce(
                        out=agg_s[:, off : off + RCH],
                        in_=m3[:, :, 0:3],
                        axis=AX.X,
                        op=OP.add,
                    ).then_inc(vs, 1)
                # post: z = relu(dinv*(agg+gprev)+b)
                for c in range(NCHUNK):
                    sl = slice(c * CHUNK, (c + 1) * CHUNK)
                    e.tensor_tensor(
                        out=z_s[:, sl], in0=agg_s[:, sl], in1=gp_s[:, sl], op=OP.add
                    )
                    e.tensor_tensor(
                        out=z_s[:, sl], in0=z_s[:, sl], in1=di_s[:, sl], op=OP.mult
                    )
                    e.tensor_scalar(
                        out=z_s[:, sl], in0=z_s[:, sl],
                        scalar1=b_s[:], scalar2=0.0, op0=OP.add, op1=OP.max,
                    ).then_inc(vs, 1)
                if w2_needed:
                    for c in range(NCHUNK):
                        sl = slice(c * CHUNK, (c + 1) * CHUNK)
                        for h in range(2):
                            e.wait_ge(ts, 2 * c + h + 1)
                            psl = slice(64 * h, 64 * (h + 1))
                            e.tensor_tensor(
                                out=go_s[psl, sl],
                                in0=ps[(2 * c + h) % 4][:],
                                in1=di_s[psl, sl],
                                op=OP.mult,
                            ).then_inc(vs, 1)

            if w2_needed:
                @blk.tensor
                def _(e):
                    e.wait_ge(dma, 80)
                    for c in range(NCHUNK):
                        e.wait_ge(vs, NRED + c + 1)
                        if c >= 2:
                            e.wait_ge(vs, NRED + NCHUNK + 2 * (c - 1))  # psum WAR
                        sl = slice(c * CHUNK, (c + 1) * CHUNK)
                        for h in range(2):
                            nc.tensor.matmul(
                                ps[(2 * c + h) % 4][:],
                                w2_s[64 * h : 64 * (h + 1), :],
                                z_s[64 * h : 64 * (h + 1), sl],
                                start=True,
                                stop=True,
                            ).then_inc(ts, 1)
    return nc


# ---------------------------------------------------------------- P4
def _build_p4(granges):
    """granges: list of (graph, half, lo, hi) reduce jobs over the global
    stacked z layout [128, GHALF] (partitions 0-63: nodes [0, GHALF),
    64-127: nodes [GHALF, 2*GHALF))."""
    nc = bacc.Bacc()
    z_i = nc.declare_dram_parameter("z", [128, GHALF], BF, isOutput=False)
    cnt = nc.declare_dram_parameter("cnt", [64, N_GRAPHS], F32, isOutput=False)
    wfc = nc.declare_dram_parameter("wfc", [64, N_CLS], BF, isOutput=False)
    bfc = nc.declare_dram_parameter("bfc", [N_CLS, 1], F32, isOutput=False)
    ident = nc.declare_dram_parameter("ident", [N_CLS, N_CLS], BF, isOutput=False)
    out_o = nc.declare_dram_parameter("out", [128, 4 * N_CLS], F32, isOutput=True)
    n_hi = sum(1 for _, h, _, _ in granges if h == 1)
    with ExitStack() as ctx:
        _n = iter(range(1000))
        sb = lambda s, d: ctx.enter_context(nc.sbuf_tensor(f"t{next(_n)}", s, d))
        z_s = sb([128, GHALF], BF)
        cnt_s = sb([64, N_GRAPHS], F32)
        wfc_s = sb([64, N_CLS], BF)
        bfc_s = sb([N_CLS, 1], F32)
        id_s = sb([N_CLS, N_CLS], BF)
        pool2_s = sb([128, N_GRAPHS], F32)   # hi-half partials on partitions 64:128
        poolhi_s = sb([64, N_GRAPHS], F32)   # hi partials moved to partitions 0:64
        pool_s = sb([64, N_GRAPHS], F32)
        poolb_s = sb([64, N_GRAPHS], BF)
        lg_s = sb([N_CLS, N_GRAPHS], BF)
        lt_s = sb([128, N_CLS], F32)
        e_s = sb([128, N_CLS], F32)
        m_s = sb([128, 1], F32)
        s_s = sb([128, 1], F32)
        out_s = sb([128, 4 * N_CLS], F32)
        ps_l = ctx.enter_context(nc.psum_tensor("ps_l", [N_CLS, N_GRAPHS], F32))
        ps_t = ctx.enter_context(nc.psum_tensor("ps_t", [128, N_CLS], BF))
        dma = ctx.enter_context(nc.semaphore("dma"))
        vs = ctx.enter_context(nc.semaphore("vs"))
        ts = ctx.enter_context(nc.semaphore("ts"))
        ss = ctx.enter_context(nc.semaphore("ss"))
        blk = ctx.enter_context(nc.Block())

        with nc.allow_low_precision("bf16 dataflow by design"):
            @blk.sync
            def _(e):
                for dst, src_ in [(z_s, z_i), (cnt_s, cnt), (wfc_s, wfc),
                                  (bfc_s, bfc), (id_s, ident)]:
                    e.dma_start(out=dst[:], in_=src_[:]).then_inc(dma, 16)
                e.wait_ge(vs, 1)  # memset+hi reduces done
                e.dma_start(out=poolhi_s[:], in_=pool2_s[64:128, :]).then_inc(dma, 16)
                e.wait_ge(vs, 15)
                e.dma_start(out=out_o[:], in_=out_s[:]).then_inc(dma, 16)
                e.wait_ge(dma, 16 * 7)

            @blk.vector
            def _(e):
                e.wait_ge(dma, 80)
                e.memset(pool_s[:], 0.0)
                e.memset(pool2_s[:], 0.0)
                last = None
                for g, h, lo, hi in granges:
                    o = pool_s if h == 0 else pool2_s
                    psl = slice(0, 64) if h == 0 else slice(64, 128)
                    last = e.tensor_reduce(
                        out=o[psl, g : g + 1] if h else o[:, g : g + 1],
                        in_=z_s[psl, lo:hi],
                        axis=AX.X,
                        op=OP.add,
                    )
                last.then_inc(vs, 1)
                e.wait_ge(dma, 96)  # poolhi moved
                e.tensor_tensor(out=pool_s[:], in0=pool_s[:], in1=poolhi_s[:], op=OP.add)
                e.reciprocal(cnt_s[:], cnt_s[:])
                e.tensor_tensor(out=pool_s[:], in0=pool_s[:], in1=cnt_s[:], op=OP.mult)
                e.tensor_copy(out=poolb_s[:], in_=pool_s[:]).then_inc(vs, 1)  # v=2
                e.wait_ge(ts, 1)
                e.tensor_scalar(
                    out=lg_s[:], in0=ps_l[:], scalar1=bfc_s[:], scalar2=None,
                    op0=OP.add,
                ).then_inc(vs, 1)  # v=3: logits bf16 ready
                for t in range(4):
                    e.wait_ge(ts, 2 + t)
                    e.tensor_copy(out=lt_s[:], in_=ps_t[:])
                    e.tensor_reduce(out=m_s[:], in_=lt_s[:], axis=AX.X, op=OP.max)
                    e.tensor_scalar(
                        out=m_s[:], in0=m_s[:], scalar1=-1.0, scalar2=None,
                        op0=OP.mult,
                    ).then_inc(vs, 1)  # neg-max ready (v=4+3t)
                    e.wait_ge(ss, 2 * t + 1)
                    e.tensor_reduce(
                        out=s_s[:], in_=e_s[:], axis=AX.X, op=OP.add
                    ).then_inc(vs, 1)  # expsum ready (v=5+3t)
                    e.wait_ge(ss, 2 * t + 2)
                    e.tensor_scalar(
                        out=s_s[:], in0=s_s[:], scalar1=-1.0, scalar2=None, op0=OP.mult
                    )
                    e.tensor_scalar(
                        out=out_s[:, t * N_CLS : (t + 1) * N_CLS],
                        in0=lt_s[:], scalar1=m_s[:], scalar2=s_s[:],
                        op0=OP.add, op1=OP.add,
                    ).then_inc(vs, 1)  # tile done (v=6+3t)

            @blk.scalar
            def _(e):
                for t in range(4):
                    e.wait_ge(vs, 4 + 3 * t)
                    e.activation(e_s[:], lt_s[:], ACT.Exp, bias=m_s[:]).then_inc(ss, 1)
                    e.wait_ge(vs, 5 + 3 * t)
                    e.activation(s_s[:], s_s[:], ACT.Ln).then_inc(ss, 1)

            @blk.tensor
            def _(e):
                e.wait_ge(vs, 2)
                nc.tensor.matmul(
                    ps_l[:], wfc_s[:], poolb_s[:], start=True, stop=True
                ).then_inc(ts, 1)
                e.wait_ge(vs, 3)
                for t in range(4):
                    if t >= 1:
                        e.wait_ge(vs, 6 + 3 * (t - 1))  # psum WAR
                    nc.tensor.matmul(
                        ps_t[:],
                        lg_s[:, 128 * t : 128 * (t + 1)],
                        id_s[:],
                        start=True,
                        stop=True,
                        is_transpose=True,
                    ).then_inc(ts, 1)
    return nc


# ---------------------------------------------------------------- host side
def _stack(a):
    """[64, n] -> [128, n//2] node-half stacking."""
    h = a.shape[1] // 2
    return np.ascontiguousarray(np.concatenate([a[:, :h], a[:, h:]], axis=0))


def kernel(x, edge_index, batch, W1, b1, W2, b2, Wfc, bfc):
    x = np.asarray(x, np.float32)
    src = np.asarray(edge_index[0], np.int64)
    dst = np.asarray(edge_index[1], np.int64)
    batch = np.asarray(batch, np.int64)
    W1 = np.asarray(W1, np.float32); b1 = np.asarray(b1, np.float32)
    W2 = np.asarray(W2, np.float32); b2 = np.asarray(b2, np.float32)
    Wfc = np.asarray(Wfc, np.float32); bfc = np.asarray(bfc, np.float32)
    _TRACE_NS.clear()
    _device_reset()

    deg = np.bincount(dst, minlength=N_NODES).astype(np.float32) + 1.0

    # --- per-core degree-sorted permutation + two-tier slot grids (integer only)
    VALID_T = (24, 48, 96)
    grids, colmaps = [], []
    t1_need, t2_need = 0, 0
    for c in range(NC):
        lo, hi = c * SH, (c + 1) * SH
        m = (dst >= lo) & (dst < hi)
        dl = dst[m] - lo
        sg = src[m]
        counts = np.bincount(dl, minlength=SHP)
        # rank nodes by degree (desc); device column order interleaves ranks
        # across the two partition half-groups so tiers align per half.
        perm = np.argsort(-counts, kind="stable")  # rank r -> local node
        colmap = np.empty(SHP, np.int64)           # stacked pos -> local node
        r = np.arange(SHP)
        colmap[(r % 2) * HALF + r // 2] = perm[r]
        colmaps.append(colmap)
        starts = np.zeros(SHP + 1, np.int64)
        np.cumsum(counts, out=starts[1:])
        order = np.argsort(dl, kind="stable")
        dls, sgs = dl[order], sg[order]
        pos = np.arange(len(dls)) - starts[dls]
        grid = np.full((SHP, 96), -1, np.int64)    # local-node-major, max 96
        grid[dls, pos] = sgs
        grids.append(grid)
        tier1 = counts[perm[: 2 * K1]]
        tier2 = counts[perm[2 * K1 :]]
        t1_need = max(t1_need, int(tier1.max(initial=0)))
        t2_need = max(t2_need, int(tier2.max(initial=0)))
    T1 = next(t for t in VALID_T if t >= t1_need)
    T2 = next(t for t in VALID_T if t >= t2_need)

    cnt = np.maximum(np.bincount(batch, minlength=N_GRAPHS), 1).astype(np.float32)

    # --- P1
    nc1 = _build_p1()
    in1 = []
    for c in range(NC):
        xs = np.zeros((SHP, D_IN), np.float32)
        xs[:SH] = x[c * SH : (c + 1) * SH]
        degs = np.ones(SHP, np.float32)
        degs[:SH] = deg[c * SH : (c + 1) * SH]
        in1.append({
            "xT": np.ascontiguousarray(xs.T).astype(BF16),
            "degT": np.ascontiguousarray(np.tile(degs[None, :], (64, 1))),
            "w1": W1.astype(BF16),
        })
    r1 = _run(nc1, in1, "p1")
    g1 = [np.asarray(r1[c]["g1"]) for c in range(NC)]
    dinv = [np.asarray(r1[c]["dinv"]) for c in range(NC)]
    g1_full = np.concatenate([g[:, :SH] for g in g1], axis=1)  # [64, 100000] bf16

    f1 = K1 * T1
    ftot = f1 + (HALF - K1) * T2

    def expand(gfull):
        gz = np.concatenate([gfull, np.zeros((64, 1), BF16)], axis=1)
        outs = []
        for c in range(NC):
            out = np.empty((128, ftot), BF16)
            for h in range(2):
                cm = colmaps[c][h * HALF : (h + 1) * HALF]
                g1r = grids[c][cm[:K1], :T1].ravel()
                g2r = grids[c][cm[K1:], :T2].ravel()
                flat = np.concatenate([g1r, g2r])
                idx = np.where(flat >= 0, flat, N_NODES)
                out[64 * h : 64 * (h + 1)] = gz[:, idx]
            outs.append(out)
        return outs

    def to_dev(a, c):
        # [64, SHP] host-local-order -> [128, HALF] device stacked order
        b = a[:, colmaps[c]]
        return np.ascontiguousarray(np.concatenate([b[:, :HALF], b[:, HALF:]], axis=0))

    def from_dev(v, c):
        # [128, HALF] device -> [64, SHP] host-local-order
        b = np.concatenate([v[0:64], v[64:128]], axis=1)
        a = np.empty_like(b)
        a[:, colmaps[c]] = b
        return a

    b1col = np.ascontiguousarray(np.tile(b1[:, None], (2, 1)).astype(np.float32))
    b2col = np.ascontiguousarray(np.tile(b2[:, None], (2, 1)).astype(np.float32))

    # --- P2
    nc2 = _build_p23(True, T1, T2)
    msgs1 = expand(g1_full)
    in2 = [{
        "msgs": msgs1[c],
        "gprev": to_dev(g1[c], c),
        "dinv": to_dev(dinv[c], c),
        "bcol": b1col,
        "w2": W2.astype(BF16),
    } for c in range(NC)]
    r2 = _run(nc2, in2, "p2")
    g2 = [from_dev(np.asarray(r2[c]["gout"]), c) for c in range(NC)]
    g2_full = np.concatenate([g[:, :SH] for g in g2], axis=1)

    # --- P3
    nc3 = _build_p23(False, T1, T2)
    msgs2 = expand(g2_full)
    in3 = [{
        "msgs": msgs2[c],
        "gprev": to_dev(g2[c], c),
        "dinv": to_dev(dinv[c], c),
        "bcol": b2col,
    } for c in range(NC)]
    r3 = _run(nc3, in3, "p3")
    z = [from_dev(np.asarray(r3[c]["gout"]), c)[:, :SH] for c in range(NC)]
    z_glob = np.concatenate(z, axis=1)  # [64, 100000] bf16, global node order

    # --- P4: per-graph ranges over the global stacked layout
    bounds = np.searchsorted(batch, np.arange(N_GRAPHS + 1))
    granges = []
    for g in range(N_GRAPHS):
        s, e = int(bounds[g]), int(bounds[g + 1])
        if s == e:
            continue
        for h in range(2):
            a, b = max(s, h * GHALF), min(e, (h + 1) * GHALF)
            if a < b:
                granges.append((g, h, a - h * GHALF, b - h * GHALF))
    nc4 = _build_p4(granges)
    zpad = np.zeros((64, 2 * GHALF), BF16)
    zpad[:, :N_NODES] = z_glob
    in4 = [{
        "z": _stack(zpad),
        "cnt": np.ascontiguousarray(np.tile(cnt[None, :], (64, 1))),
        "wfc": Wfc.astype(BF16),
        "bfc": np.ascontiguousarray(bfc[:, None].astype(np.float32)),
        "ident": np.eye(N_CLS, dtype=BF16),
    }] * NC
    r4 = _run(nc4, in4, "p4")
    o = np.asarray(r4[0]["out"])  # [128, 40]
    out = np.zeros((N_GRAPHS, N_CLS), np.float32)
    for t in range(4):
        out[t * 128 : (t + 1) * 128] = o[:, t * N_CLS : (t + 1) * N_CLS]
    return out


kernel.trace_ns = _TRACE_NS
